# revision 10
# baseline (speedup 1.0000x reference)
"""Block-sparse 3-layer MLP on 8 Trainium2 NeuronCores.

Reference computation (fp32):
    h1 = relu(x @ (W1*expand(mask1)).T + b1)       x:[B,2048] W1:[4096,2048]
    h2 = relu(h1 @ (W2*expand(mask2)).T + b2)      W2:[4096,4096]
    out = h2 @ Wo.T + bo                           Wo:[1024,4096] -> [B,1024]

Strategy: data-parallel over the batch (B=8192 -> 1024 rows/core), no
collectives. Masks are applied to the weights on the host (free), and all
matmuls run dense on the PE array. Activations are kept feature-major
[features, batch] on-chip so biases are per-partition and `lhsT` panels are
pre-transposed on the host into contiguous [128, K] blocks.

Per core:
  L1: h1 (32 tiles [128,1024]) stays resident in SBUF.
  L2+L3 fused: for each of W2's 32 row-blocks, compute h2 tile, immediately
  multiply against Wo panels, accumulate the [1024,1024] output in SBUF via
  DVE adds. No intermediate ever touches DRAM; W1/W2/Wo are each read once.

MM_DTYPE selects the PE datapath: fp32 is exact but 4 cycles/row on the PE;
bf16 and float32r stream at 1 cycle/row (4x faster).
"""

import sys

sys.path.insert(0, "/opt/trn_rl_repo")

import numpy as np

from concourse import bacc, mybir, tile
from concourse.bass_utils import run_bass_kernel_spmd

F32 = mybir.dt.float32
F32R = mybir.dt.float32r
BF16 = mybir.dt.bfloat16
RELU = mybir.ActivationFunctionType.Relu
IDENT = mybir.ActivationFunctionType.Identity

N_CORES = 8
TILE = 32  # block-sparse tile size of the masks
P = 128  # partitions

MM_DTYPE = "f32r"  # "f32" | "f32r" | "bf16"


def _build(nc, d_in, d_h, d_out, bc, mm_dtype=MM_DTYPE):
    """Emit the per-core kernel. bc = batch columns per core."""
    kt1 = d_in // P  # k-tiles in layer 1
    mt1 = d_h // P  # m-tiles of h1 (== k-tiles of layer 2)
    mt2 = d_h // P  # m-tiles of h2
    mot = d_out // P  # m-tiles of out
    sw = min(512, bc)  # psum strip width
    ns = bc // sw  # strips per row of tiles

    # storage dtype of mm operands
    sdt = {"bf16": BF16, "f32r": F32R, "f32": F32}[mm_dtype]

    def mm(out_ps, lhsT, rhs, start, stop):
        nc.tensor.matmul(out_ps, lhsT, rhs, start=start, stop=stop)

    xt_d = nc.dram_tensor("xt", [kt1, P, bc], sdt, kind="ExternalInput")
    w1_d = nc.dram_tensor("w1", [mt1, P, d_in], sdt, kind="ExternalInput")
    b1_d = nc.dram_tensor("b1", [P, mt1], F32, kind="ExternalInput")
    w2_d = nc.dram_tensor("w2", [mt2, P, d_h], sdt, kind="ExternalInput")
    b2_d = nc.dram_tensor("b2", [P, mt2], F32, kind="ExternalInput")
    wo_d = nc.dram_tensor("wo", [mt2, P, d_out], sdt, kind="ExternalInput")
    bo_d = nc.dram_tensor("bo", [P, mot], F32, kind="ExternalInput")
    out_d = nc.dram_tensor("out", [mot, P, bc], F32, kind="ExternalOutput")

    with tile.TileContext(nc) as tc:
        with (
            tc.tile_pool(name="bias", bufs=1) as bias_pool,
            tc.tile_pool(name="h1", bufs=1) as h1_pool,
        ):
            b1_sb = bias_pool.tile([P, mt1], F32, tag="b1")
            b2_sb = bias_pool.tile([P, mt2], F32, tag="b2")
            bo_sb = bias_pool.tile([P, mot], F32, tag="bo")
            nc.sync.dma_start(out=b1_sb[:], in_=b1_d[:])
            nc.sync.dma_start(out=b2_sb[:], in_=b2_d[:])
            nc.sync.dma_start(out=bo_sb[:], in_=bo_d[:])

            h1 = []
            for i in range(mt1):
                t = h1_pool.tile([P, bc], sdt, name=f"h1_{i}", tag=f"h1_{i}")
                h1.append(t)

            # ---------------- Layer 1 ----------------
            with (
                tc.tile_pool(name="xtp", bufs=1) as xt_pool,
                tc.tile_pool(name="w1p", bufs=6) as w1_pool,
                tc.tile_pool(name="ps1", bufs=4, space="PSUM") as ps1_pool,
            ):
                xt = []
                for kt in range(kt1):
                    t = xt_pool.tile([P, bc], sdt, name=f"xt_{kt}", tag=f"xt_{kt}")
                    nc.sync.dma_start(out=t[:], in_=xt_d[kt])
                    xt.append(t)

                # stream each weight panel in quarters so the pool stays
                # small enough to double-buffer within SBUF
                kh1 = max(kt1 // 4, 1)  # k-tiles per panel piece
                for mt in range(mt1):
                    w1h = []
                    for h in range(kt1 // kh1):
                        t = w1_pool.tile([P, kh1 * P], sdt, tag="w1t")
                        nc.sync.dma_start(
                            out=t[:], in_=w1_d[mt][:, h * kh1 * P : (h + 1) * kh1 * P]
                        )
                        w1h.append(t)
                    for n in range(ns):
                        ps = ps1_pool.tile([P, sw], F32, tag="ps1")
                        cs = slice(n * sw, (n + 1) * sw)
                        for kt in range(kt1):
                            h, r = divmod(kt, kh1)
                            mm(
                                ps[:],
                                w1h[h][:, r * P : (r + 1) * P],
                                xt[kt][:, cs],
                                start=(kt == 0),
                                stop=(kt == kt1 - 1),
                            )
                        nc.scalar.activation(
                            h1[mt][:, cs], ps[:], RELU, bias=b1_sb[:, mt : mt + 1]
                        )

            # ---------------- Layers 2 + 3 (fused) ----------------
            with (
                tc.tile_pool(name="w2p", bufs=6) as w2_pool,
                tc.tile_pool(name="wop", bufs=2) as wo_pool,
                tc.tile_pool(name="h2p", bufs=2) as h2_pool,
                tc.tile_pool(name="oacc", bufs=1) as oacc_pool,
                tc.tile_pool(name="ps2", bufs=4, space="PSUM") as ps2_pool,
                tc.tile_pool(name="ps3", bufs=4, space="PSUM") as ps3_pool,
            ):
                oacc = []
                for i in range(mot):
                    t = oacc_pool.tile([P, bc], F32, name=f"oacc_{i}", tag=f"oacc_{i}")
                    oacc.append(t)

                kh2 = max(mt1 // 4, 1)
                for mt in range(mt2):
                    w2h = []
                    for h in range(mt1 // kh2):
                        t = w2_pool.tile([P, kh2 * P], sdt, tag="w2t")
                        nc.sync.dma_start(
                            out=t[:], in_=w2_d[mt][:, h * kh2 * P : (h + 1) * kh2 * P]
                        )
                        w2h.append(t)
                    wot = wo_pool.tile([P, d_out], sdt, tag="wot")
                    nc.sync.dma_start(out=wot[:], in_=wo_d[mt])
                    h2t = h2_pool.tile([P, bc], sdt, tag="h2t")
                    for n in range(ns):
                        ps = ps2_pool.tile([P, sw], F32, tag="ps2")
                        cs = slice(n * sw, (n + 1) * sw)
                        for kt in range(mt1):
                            h, r = divmod(kt, kh2)
                            mm(
                                ps[:],
                                w2h[h][:, r * P : (r + 1) * P],
                                h1[kt][:, cs],
                                start=(kt == 0),
                                stop=(kt == mt1 - 1),
                            )
                        nc.scalar.activation(
                            h2t[:, cs], ps[:], RELU, bias=b2_sb[:, mt : mt + 1]
                        )
                        for mo in range(mot):
                            ps3 = ps3_pool.tile([P, sw], F32, tag="ps3")
                            mm(
                                ps3[:],
                                wot[:, mo * P : (mo + 1) * P],
                                h2t[:, cs],
                                start=True,
                                stop=True,
                            )
                            if mt == 0:
                                nc.vector.tensor_copy(oacc[mo][:, cs], ps3[:])
                            else:
                                nc.vector.tensor_add(
                                    oacc[mo][:, cs], oacc[mo][:, cs], ps3[:]
                                )

                for mo in range(mot):
                    nc.scalar.activation(
                        oacc[mo][:], oacc[mo][:], IDENT, bias=bo_sb[:, mo : mo + 1]
                    )
                    nc.sync.dma_start(out=out_d[mo], in_=oacc[mo][:])

    nc.compile()
    return nc


def _expand_mask(mask, t=TILE):
    return np.repeat(np.repeat(np.asarray(mask, dtype=bool), t, axis=0), t, axis=1)


def _pack_lhsT(w, d_m, d_k):
    """[d_m, d_k] weights -> [d_m/P, P, d_k] panels.

    panel[mt, i, kt*P + j] = w[mt*P + j, kt*P + i], so each [P, P] slice of a
    panel is a ready-to-use lhsT block (partition dim = contraction dim).
    """
    mt, kt = d_m // P, d_k // P
    return np.ascontiguousarray(
        w.reshape(mt, P, kt, P).transpose(0, 3, 2, 1).reshape(mt, P, d_k)
    )


def _pack_out_panels(w, d_m, d_k):
    """[d_m, d_k] weights -> [d_k/P, P, d_m] panels keyed by the k-tile.

    panel[kt, i, mo*P + j] = w[mo*P + j, kt*P + i].
    """
    mt, kt = d_m // P, d_k // P
    return np.ascontiguousarray(
        w.reshape(mt, P, kt, P).transpose(2, 3, 0, 1).reshape(kt, P, d_m)
    )


def _pack_bias(b):
    n = b.shape[0] // P
    return np.ascontiguousarray(b.reshape(n, P).T)


def _run(
    x,
    w1e,
    b1,
    w2e,
    b2,
    wo,
    bo,
    d_in,
    d_h,
    d_out,
    n_cores=N_CORES,
    trace=False,
    mm_dtype=MM_DTYPE,
):
    b = x.shape[0]
    bc = b // n_cores

    nc = bacc.Bacc(
        "TRN2", target_bir_lowering=False, debug=False, num_devices=n_cores
    )
    _build(nc, d_in, d_h, d_out, bc, mm_dtype=mm_dtype)

    np_sdt = mybir.dt.np(BF16) if mm_dtype == "bf16" else np.float32

    def cvt(a):
        return np.ascontiguousarray(a.astype(np_sdt))

    shared = {
        "w1": cvt(_pack_lhsT(w1e, d_h, d_in)),
        "b1": _pack_bias(b1),
        "w2": cvt(_pack_lhsT(w2e, d_h, d_h)),
        "b2": _pack_bias(b2),
        "wo": cvt(_pack_out_panels(wo, d_out, d_h)),
        "bo": _pack_bias(bo),
    }
    in_maps = []
    for c in range(n_cores):
        xc = np.ascontiguousarray(x[c * bc : (c + 1) * bc].T).reshape(
            d_in // P, P, bc
        )
        in_maps.append({"xt": cvt(xc), **shared})

    res = run_bass_kernel_spmd(
        nc, in_maps, core_ids=list(range(n_cores)), trace=trace
    )
    outs = []
    for c in range(n_cores):
        outs.append(res.results[c]["out"].reshape(d_out, bc))
    full = np.concatenate(outs, axis=1)  # [d_out, B]
    return np.ascontiguousarray(full.T), res


def kernel(x, W1, b1, W2, b2, Wo, bo, mask1, mask2):
    x = np.asarray(x, dtype=np.float32)
    w1e = np.asarray(W1, dtype=np.float32) * _expand_mask(mask1)
    w2e = np.asarray(W2, dtype=np.float32) * _expand_mask(mask2)
    out, _ = _run(
        x,
        w1e,
        np.asarray(b1, np.float32),
        w2e,
        np.asarray(b2, np.float32),
        np.asarray(Wo, np.float32),
        np.asarray(bo, np.float32),
        d_in=2048,
        d_h=4096,
        d_out=1024,
    )
    return out


# revision 14
# speedup vs baseline: 1.0300x; 1.0300x over previous
"""Block-sparse 3-layer MLP on 8 Trainium2 NeuronCores.

Reference computation (fp32):
    h1 = relu(x @ (W1*expand(mask1)).T + b1)       x:[B,2048] W1:[4096,2048]
    h2 = relu(h1 @ (W2*expand(mask2)).T + b2)      W2:[4096,4096]
    out = h2 @ Wo.T + bo                           Wo:[1024,4096] -> [B,1024]

Strategy: data-parallel over the batch (B=8192 -> 1024 rows/core), no
collectives. Masks are applied to the weights on the host (free), and all
matmuls run dense on the PE array. Activations are kept feature-major
[features, batch] on-chip so biases are per-partition and `lhsT` panels are
pre-transposed on the host into contiguous [128, K] blocks.

Per core:
  L1: h1 (32 tiles [128,1024]) stays resident in SBUF.
  L2+L3 fused: for each of W2's 32 row-blocks, compute h2 tile, immediately
  multiply against Wo panels, accumulate the [1024,1024] output in SBUF via
  DVE adds. No intermediate ever touches DRAM; W1/W2/Wo are each read once.

MM_DTYPE selects the PE datapath: fp32 is exact but 4 cycles/row on the PE;
bf16 and float32r stream at 1 cycle/row (4x faster).
"""

import sys

sys.path.insert(0, "/opt/trn_rl_repo")

import numpy as np

from concourse import bacc, mybir, tile
from concourse.bass_utils import run_bass_kernel_spmd

F32 = mybir.dt.float32
F32R = mybir.dt.float32r
BF16 = mybir.dt.bfloat16
RELU = mybir.ActivationFunctionType.Relu
IDENT = mybir.ActivationFunctionType.Identity

N_CORES = 8
TILE = 32  # block-sparse tile size of the masks
P = 128  # partitions

MM_DTYPE = "f32r"  # "f32" | "f32r" | "bf16"


def _build(nc, d_in, d_h, d_out, bc, mm_dtype=MM_DTYPE):
    """Emit the per-core kernel. bc = batch columns per core."""
    kt1 = d_in // P  # k-tiles in layer 1
    mt1 = d_h // P  # m-tiles of h1 (== k-tiles of layer 2)
    mt2 = d_h // P  # m-tiles of h2
    mot = d_out // P  # m-tiles of out
    sw = min(512, bc)  # psum strip width
    ns = bc // sw  # strips per row of tiles

    # storage dtype of mm operands
    sdt = {"bf16": BF16, "f32r": F32R, "f32": F32}[mm_dtype]

    def mm(out_ps, lhsT, rhs, start, stop):
        nc.tensor.matmul(out_ps, lhsT, rhs, start=start, stop=stop)

    xt_d = nc.dram_tensor("xt", [kt1, P, bc], sdt, kind="ExternalInput")
    w1_d = nc.dram_tensor("w1", [mt1, P, d_in], sdt, kind="ExternalInput")
    b1_d = nc.dram_tensor("b1", [P, mt1], F32, kind="ExternalInput")
    w2_d = nc.dram_tensor("w2", [mt2, P, d_h], sdt, kind="ExternalInput")
    b2_d = nc.dram_tensor("b2", [P, mt2], F32, kind="ExternalInput")
    wo_d = nc.dram_tensor("wo", [mt2, P, d_out], sdt, kind="ExternalInput")
    bo_d = nc.dram_tensor("bo", [P, mot], F32, kind="ExternalInput")
    out_d = nc.dram_tensor("out", [mot, P, bc], F32, kind="ExternalOutput")

    with tile.TileContext(nc) as tc:
        with (
            tc.tile_pool(name="bias", bufs=1) as bias_pool,
            tc.tile_pool(name="h1", bufs=1) as h1_pool,
        ):
            b1_sb = bias_pool.tile([P, mt1], F32, tag="b1")
            b2_sb = bias_pool.tile([P, mt2], F32, tag="b2")
            bo_sb = bias_pool.tile([P, mot], F32, tag="bo")
            nc.sync.dma_start(out=b1_sb[:], in_=b1_d[:])
            nc.sync.dma_start(out=b2_sb[:], in_=b2_d[:])
            nc.sync.dma_start(out=bo_sb[:], in_=bo_d[:])

            h1 = []
            for i in range(mt1):
                t = h1_pool.tile([P, bc], sdt, name=f"h1_{i}", tag=f"h1_{i}")
                h1.append(t)

            # ---------------- Layer 1 ----------------
            with (
                tc.tile_pool(name="xtp", bufs=1) as xt_pool,
                tc.tile_pool(name="w1p", bufs=6) as w1_pool,
                tc.tile_pool(name="ps1", bufs=4, space="PSUM") as ps1_pool,
            ):
                # stream each weight panel in quarters so the pool stays
                # small enough to double-buffer within SBUF
                kh1 = max(kt1 // 4, 1)  # k-tiles per panel piece

                def load_w1(mt):
                    w1h = []
                    for h in range(kt1 // kh1):
                        t = w1_pool.tile([P, kh1 * P], sdt, tag="w1t")
                        nc.sync.dma_start(
                            out=t[:], in_=w1_d[mt][:, h * kh1 * P : (h + 1) * kh1 * P]
                        )
                        w1h.append(t)
                    return w1h

                # first panel goes ahead of the xt loads so the PE can start
                # as soon as xt_0 lands instead of after the whole xt stream
                w1h0 = load_w1(0)
                xt = []
                for kt in range(kt1):
                    t = xt_pool.tile([P, bc], sdt, name=f"xt_{kt}", tag=f"xt_{kt}")
                    nc.sync.dma_start(out=t[:], in_=xt_d[kt])
                    xt.append(t)

                for mt in range(mt1):
                    w1h = w1h0 if mt == 0 else load_w1(mt)
                    # weight-stationary inner order: each lhsT block feeds
                    # every batch strip before the next LDWEIGHTS
                    pss = [ps1_pool.tile([P, sw], F32, name="ps1", tag="ps1") for _ in range(ns)]
                    for kt in range(kt1):
                        h, r = divmod(kt, kh1)
                        for n in range(ns):
                            mm(
                                pss[n][:],
                                w1h[h][:, r * P : (r + 1) * P],
                                xt[kt][:, n * sw : (n + 1) * sw],
                                start=(kt == 0),
                                stop=(kt == kt1 - 1),
                            )
                    for n in range(ns):
                        cs = slice(n * sw, (n + 1) * sw)
                        nc.scalar.activation(
                            h1[mt][:, cs], pss[n][:], RELU, bias=b1_sb[:, mt : mt + 1]
                        )

            # ---------------- Layers 2 + 3 (fused) ----------------
            with (
                tc.tile_pool(name="w2p", bufs=6) as w2_pool,
                tc.tile_pool(name="wop", bufs=2) as wo_pool,
                tc.tile_pool(name="h2p", bufs=2) as h2_pool,
                tc.tile_pool(name="oacc", bufs=1) as oacc_pool,
                tc.tile_pool(name="ps2", bufs=4, space="PSUM") as ps2_pool,
                tc.tile_pool(name="ps3", bufs=4, space="PSUM") as ps3_pool,
            ):
                oacc = []
                for i in range(mot):
                    t = oacc_pool.tile([P, bc], F32, name=f"oacc_{i}", tag=f"oacc_{i}")
                    oacc.append(t)

                kh2 = max(mt1 // 4, 1)
                for mt in range(mt2):
                    w2h = []
                    for h in range(mt1 // kh2):
                        t = w2_pool.tile([P, kh2 * P], sdt, tag="w2t")
                        nc.sync.dma_start(
                            out=t[:], in_=w2_d[mt][:, h * kh2 * P : (h + 1) * kh2 * P]
                        )
                        w2h.append(t)
                    wot = wo_pool.tile([P, d_out], sdt, tag="wot")
                    nc.sync.dma_start(out=wot[:], in_=wo_d[mt])
                    h2t = h2_pool.tile([P, bc], sdt, tag="h2t")
                    pss = [ps2_pool.tile([P, sw], F32, name="ps2", tag="ps2") for _ in range(ns)]
                    for kt in range(mt1):
                        h, r = divmod(kt, kh2)
                        for n in range(ns):
                            mm(
                                pss[n][:],
                                w2h[h][:, r * P : (r + 1) * P],
                                h1[kt][:, n * sw : (n + 1) * sw],
                                start=(kt == 0),
                                stop=(kt == mt1 - 1),
                            )
                    for n in range(ns):
                        cs = slice(n * sw, (n + 1) * sw)
                        nc.scalar.activation(
                            h2t[:, cs], pss[n][:], RELU, bias=b2_sb[:, mt : mt + 1]
                        )
                    last = mt == mt2 - 1 and mt2 > 1
                    for mo in range(mot):
                        for n in range(ns):
                            cs = slice(n * sw, (n + 1) * sw)
                            ps3 = ps3_pool.tile([P, sw], F32, tag="ps3")
                            mm(
                                ps3[:],
                                wot[:, mo * P : (mo + 1) * P],
                                h2t[:, cs],
                                start=True,
                                stop=True,
                            )
                            if mt == 0:
                                nc.vector.tensor_copy(oacc[mo][:, cs], ps3[:])
                            elif last:
                                # fold the output bias into the final
                                # accumulation: out = (ps3*1 + bo) + oacc
                                nc.vector.affine_then_add(
                                    oacc[mo][:, cs],
                                    ps3[:],
                                    oacc[mo][:, cs],
                                    1.0,
                                    bo_sb[:, mo : mo + 1],
                                )
                            else:
                                nc.vector.tensor_add(
                                    oacc[mo][:, cs], oacc[mo][:, cs], ps3[:]
                                )
                        if last:
                            nc.sync.dma_start(out=out_d[mo], in_=oacc[mo][:])

                if mt2 == 1:
                    for mo in range(mot):
                        nc.scalar.activation(
                            oacc[mo][:], oacc[mo][:], IDENT, bias=bo_sb[:, mo : mo + 1]
                        )
                        nc.sync.dma_start(out=out_d[mo], in_=oacc[mo][:])

    nc.compile()
    return nc


def _expand_mask(mask, t=TILE):
    return np.repeat(np.repeat(np.asarray(mask, dtype=bool), t, axis=0), t, axis=1)


def _pack_lhsT(w, d_m, d_k):
    """[d_m, d_k] weights -> [d_m/P, P, d_k] panels.

    panel[mt, i, kt*P + j] = w[mt*P + j, kt*P + i], so each [P, P] slice of a
    panel is a ready-to-use lhsT block (partition dim = contraction dim).
    """
    mt, kt = d_m // P, d_k // P
    return np.ascontiguousarray(
        w.reshape(mt, P, kt, P).transpose(0, 3, 2, 1).reshape(mt, P, d_k)
    )


def _pack_out_panels(w, d_m, d_k):
    """[d_m, d_k] weights -> [d_k/P, P, d_m] panels keyed by the k-tile.

    panel[kt, i, mo*P + j] = w[mo*P + j, kt*P + i].
    """
    mt, kt = d_m // P, d_k // P
    return np.ascontiguousarray(
        w.reshape(mt, P, kt, P).transpose(2, 3, 0, 1).reshape(kt, P, d_m)
    )


def _pack_bias(b):
    n = b.shape[0] // P
    return np.ascontiguousarray(b.reshape(n, P).T)


def _run(
    x,
    w1e,
    b1,
    w2e,
    b2,
    wo,
    bo,
    d_in,
    d_h,
    d_out,
    n_cores=N_CORES,
    trace=False,
    mm_dtype=MM_DTYPE,
):
    b = x.shape[0]
    bc = b // n_cores

    nc = bacc.Bacc(
        "TRN2", target_bir_lowering=False, debug=False, num_devices=n_cores
    )
    _build(nc, d_in, d_h, d_out, bc, mm_dtype=mm_dtype)

    np_sdt = mybir.dt.np(BF16) if mm_dtype == "bf16" else np.float32

    def cvt(a):
        return np.ascontiguousarray(a.astype(np_sdt))

    shared = {
        "w1": cvt(_pack_lhsT(w1e, d_h, d_in)),
        "b1": _pack_bias(b1),
        "w2": cvt(_pack_lhsT(w2e, d_h, d_h)),
        "b2": _pack_bias(b2),
        "wo": cvt(_pack_out_panels(wo, d_out, d_h)),
        "bo": _pack_bias(bo),
    }
    in_maps = []
    for c in range(n_cores):
        xc = np.ascontiguousarray(x[c * bc : (c + 1) * bc].T).reshape(
            d_in // P, P, bc
        )
        in_maps.append({"xt": cvt(xc), **shared})

    res = run_bass_kernel_spmd(
        nc, in_maps, core_ids=list(range(n_cores)), trace=trace
    )
    outs = []
    for c in range(n_cores):
        outs.append(res.results[c]["out"].reshape(d_out, bc))
    full = np.concatenate(outs, axis=1)  # [d_out, B]
    return np.ascontiguousarray(full.T), res


def kernel(x, W1, b1, W2, b2, Wo, bo, mask1, mask2):
    x = np.asarray(x, dtype=np.float32)
    w1e = np.asarray(W1, dtype=np.float32) * _expand_mask(mask1)
    w2e = np.asarray(W2, dtype=np.float32) * _expand_mask(mask2)
    out, _ = _run(
        x,
        w1e,
        np.asarray(b1, np.float32),
        w2e,
        np.asarray(b2, np.float32),
        np.asarray(Wo, np.float32),
        np.asarray(bo, np.float32),
        d_in=2048,
        d_h=4096,
        d_out=1024,
    )
    return out


# revision 17
# speedup vs baseline: 1.0419x; 1.0116x over previous
"""Block-sparse 3-layer MLP on 8 Trainium2 NeuronCores.

Reference computation (fp32):
    h1 = relu(x @ (W1*expand(mask1)).T + b1)       x:[B,2048] W1:[4096,2048]
    h2 = relu(h1 @ (W2*expand(mask2)).T + b2)      W2:[4096,4096]
    out = h2 @ Wo.T + bo                           Wo:[1024,4096] -> [B,1024]

Strategy: data-parallel over the batch (B=8192 -> 1024 rows/core), no
collectives. Masks are applied to the weights on the host (free), and all
matmuls run dense on the PE array. Activations are kept feature-major
[features, batch] on-chip so biases are per-partition and `lhsT` panels are
pre-transposed on the host into contiguous [128, K] blocks.

Per core:
  L1: h1 (32 tiles [128,1024]) stays resident in SBUF.
  L2+L3 fused: for each of W2's 32 row-blocks, compute h2 tile, immediately
  multiply against Wo panels, accumulate the [1024,1024] output in SBUF via
  DVE adds. No intermediate ever touches DRAM; W1/W2/Wo are each read once.

MM_DTYPE selects the PE datapath: fp32 is exact but 4 cycles/row on the PE;
bf16 and float32r stream at 1 cycle/row (4x faster).
"""

import sys

sys.path.insert(0, "/opt/trn_rl_repo")

import numpy as np

from concourse import bacc, mybir, tile
from concourse.bass_utils import run_bass_kernel_spmd

F32 = mybir.dt.float32
F32R = mybir.dt.float32r
BF16 = mybir.dt.bfloat16
RELU = mybir.ActivationFunctionType.Relu
IDENT = mybir.ActivationFunctionType.Identity

N_CORES = 8
TILE = 32  # block-sparse tile size of the masks
P = 128  # partitions

MM_DTYPE = "f32r"  # "f32" | "f32r" | "bf16"


def _build(nc, d_in, d_h, d_out, bc, mm_dtype=MM_DTYPE):
    """Emit the per-core kernel. bc = batch columns per core."""
    kt1 = d_in // P  # k-tiles in layer 1
    mt1 = d_h // P  # m-tiles of h1 (== k-tiles of layer 2)
    mt2 = d_h // P  # m-tiles of h2
    mot = d_out // P  # m-tiles of out
    sw = min(512, bc)  # psum strip width
    ns = bc // sw  # strips per row of tiles

    # storage dtype of mm operands
    sdt = {"bf16": BF16, "f32r": F32R, "f32": F32}[mm_dtype]

    def mm(out_ps, lhsT, rhs, start, stop):
        nc.tensor.matmul(out_ps, lhsT, rhs, start=start, stop=stop)

    xt_d = nc.dram_tensor("xt", [kt1, P, bc], sdt, kind="ExternalInput")
    w1_d = nc.dram_tensor("w1", [mt1, P, d_in], sdt, kind="ExternalInput")
    b1_d = nc.dram_tensor("b1", [P, mt1], F32, kind="ExternalInput")
    w2_d = nc.dram_tensor("w2", [mt2, P, d_h], sdt, kind="ExternalInput")
    b2_d = nc.dram_tensor("b2", [P, mt2], F32, kind="ExternalInput")
    wo_d = nc.dram_tensor("wo", [mt2, P, d_out], sdt, kind="ExternalInput")
    bo_d = nc.dram_tensor("bo", [P, mot], F32, kind="ExternalInput")
    out_d = nc.dram_tensor("out", [mot, P, bc], F32, kind="ExternalOutput")

    with tile.TileContext(nc) as tc:
        with (
            tc.tile_pool(name="bias", bufs=1) as bias_pool,
            tc.tile_pool(name="h1", bufs=1) as h1_pool,
        ):
            b1_sb = bias_pool.tile([P, mt1], F32, tag="b1")
            b2_sb = bias_pool.tile([P, mt2], F32, tag="b2")
            bo_sb = bias_pool.tile([P, mot], F32, tag="bo")
            nc.sync.dma_start(out=b1_sb[:], in_=b1_d[:])
            nc.sync.dma_start(out=b2_sb[:], in_=b2_d[:])
            nc.sync.dma_start(out=bo_sb[:], in_=bo_d[:])

            h1 = []
            for i in range(mt1):
                t = h1_pool.tile([P, bc], sdt, name=f"h1_{i}", tag=f"h1_{i}")
                h1.append(t)

            # ---------------- Layer 1 ----------------
            with (
                tc.tile_pool(name="xtp", bufs=1) as xt_pool,
                tc.tile_pool(name="w1p", bufs=6) as w1_pool,
                tc.tile_pool(name="ps1", bufs=4, space="PSUM") as ps1_pool,
            ):
                # stream each weight panel in quarters so the pool stays
                # small enough to double-buffer within SBUF
                kh1 = max(kt1 // 4, 1)  # k-tiles per panel piece

                def load_w1(mt):
                    w1h = []
                    for h in range(kt1 // kh1):
                        t = w1_pool.tile([P, kh1 * P], sdt, tag="w1t")
                        nc.sync.dma_start(
                            out=t[:], in_=w1_d[mt][:, h * kh1 * P : (h + 1) * kh1 * P]
                        )
                        w1h.append(t)
                    return w1h

                # first panel goes ahead of the xt loads so the PE can start
                # as soon as xt_0 lands instead of after the whole xt stream
                w1h0 = load_w1(0)
                # xt goes through the gpsimd (SWDGE) queues so it streams in
                # parallel with the weight panels on the sync HWDGE ring
                xt = []
                for kt in range(kt1):
                    t = xt_pool.tile([P, bc], sdt, name=f"xt_{kt}", tag=f"xt_{kt}")
                    nc.gpsimd.dma_start(out=t[:], in_=xt_d[kt])
                    xt.append(t)

                for mt in range(mt1):
                    w1h = w1h0 if mt == 0 else load_w1(mt)
                    # weight-stationary inner order: each lhsT block feeds
                    # every batch strip before the next LDWEIGHTS
                    pss = [ps1_pool.tile([P, sw], F32, name="ps1", tag="ps1") for _ in range(ns)]
                    for kt in range(kt1):
                        h, r = divmod(kt, kh1)
                        for n in range(ns):
                            mm(
                                pss[n][:],
                                w1h[h][:, r * P : (r + 1) * P],
                                xt[kt][:, n * sw : (n + 1) * sw],
                                start=(kt == 0),
                                stop=(kt == kt1 - 1),
                            )
                    for n in range(ns):
                        cs = slice(n * sw, (n + 1) * sw)
                        nc.scalar.activation(
                            h1[mt][:, cs], pss[n][:], RELU, bias=b1_sb[:, mt : mt + 1]
                        )

            # ---------------- Layers 2 + 3 (fused) ----------------
            with (
                tc.tile_pool(name="w2p", bufs=10) as w2_pool,
                tc.tile_pool(name="wop", bufs=2) as wo_pool,
                tc.tile_pool(name="h2p", bufs=2) as h2_pool,
                tc.tile_pool(name="oacc", bufs=1) as oacc_pool,
                tc.tile_pool(name="ps2", bufs=4, space="PSUM") as ps2_pool,
                tc.tile_pool(name="ps3", bufs=4, space="PSUM") as ps3_pool,
            ):
                oacc = []
                for i in range(mot):
                    t = oacc_pool.tile([P, bc], F32, name=f"oacc_{i}", tag=f"oacc_{i}")
                    oacc.append(t)

                # eighth-panels: the first piece of the next phase/panel is
                # small, so the PE never waits long at panel boundaries
                kh2 = max(mt1 // 8, 1)
                for mt in range(mt2):
                    w2h = []
                    for h in range(mt1 // kh2):
                        t = w2_pool.tile([P, kh2 * P], sdt, tag="w2t")
                        nc.sync.dma_start(
                            out=t[:], in_=w2_d[mt][:, h * kh2 * P : (h + 1) * kh2 * P]
                        )
                        w2h.append(t)
                    wot = wo_pool.tile([P, d_out], sdt, tag="wot")
                    nc.sync.dma_start(out=wot[:], in_=wo_d[mt])
                    h2t = h2_pool.tile([P, bc], sdt, tag="h2t")
                    pss = [ps2_pool.tile([P, sw], F32, name="ps2", tag="ps2") for _ in range(ns)]
                    for kt in range(mt1):
                        h, r = divmod(kt, kh2)
                        for n in range(ns):
                            mm(
                                pss[n][:],
                                w2h[h][:, r * P : (r + 1) * P],
                                h1[kt][:, n * sw : (n + 1) * sw],
                                start=(kt == 0),
                                stop=(kt == mt1 - 1),
                            )
                    for n in range(ns):
                        cs = slice(n * sw, (n + 1) * sw)
                        nc.scalar.activation(
                            h2t[:, cs], pss[n][:], RELU, bias=b2_sb[:, mt : mt + 1]
                        )
                    last = mt == mt2 - 1 and mt2 > 1
                    for mo in range(mot):
                        for n in range(ns):
                            cs = slice(n * sw, (n + 1) * sw)
                            ps3 = ps3_pool.tile([P, sw], F32, tag="ps3")
                            mm(
                                ps3[:],
                                wot[:, mo * P : (mo + 1) * P],
                                h2t[:, cs],
                                start=True,
                                stop=True,
                            )
                            if mt == 0:
                                nc.vector.tensor_copy(oacc[mo][:, cs], ps3[:])
                            elif last:
                                # fold the output bias into the final
                                # accumulation: out = (ps3*1 + bo) + oacc
                                nc.vector.affine_then_add(
                                    oacc[mo][:, cs],
                                    ps3[:],
                                    oacc[mo][:, cs],
                                    1.0,
                                    bo_sb[:, mo : mo + 1],
                                )
                            else:
                                nc.vector.tensor_add(
                                    oacc[mo][:, cs], oacc[mo][:, cs], ps3[:]
                                )
                        if last:
                            nc.sync.dma_start(out=out_d[mo], in_=oacc[mo][:])

                if mt2 == 1:
                    for mo in range(mot):
                        nc.scalar.activation(
                            oacc[mo][:], oacc[mo][:], IDENT, bias=bo_sb[:, mo : mo + 1]
                        )
                        nc.sync.dma_start(out=out_d[mo], in_=oacc[mo][:])

    nc.compile()
    return nc


def _expand_mask(mask, t=TILE):
    return np.repeat(np.repeat(np.asarray(mask, dtype=bool), t, axis=0), t, axis=1)


def _pack_lhsT(w, d_m, d_k):
    """[d_m, d_k] weights -> [d_m/P, P, d_k] panels.

    panel[mt, i, kt*P + j] = w[mt*P + j, kt*P + i], so each [P, P] slice of a
    panel is a ready-to-use lhsT block (partition dim = contraction dim).
    """
    mt, kt = d_m // P, d_k // P
    return np.ascontiguousarray(
        w.reshape(mt, P, kt, P).transpose(0, 3, 2, 1).reshape(mt, P, d_k)
    )


def _pack_out_panels(w, d_m, d_k):
    """[d_m, d_k] weights -> [d_k/P, P, d_m] panels keyed by the k-tile.

    panel[kt, i, mo*P + j] = w[mo*P + j, kt*P + i].
    """
    mt, kt = d_m // P, d_k // P
    return np.ascontiguousarray(
        w.reshape(mt, P, kt, P).transpose(2, 3, 0, 1).reshape(kt, P, d_m)
    )


def _pack_bias(b):
    n = b.shape[0] // P
    return np.ascontiguousarray(b.reshape(n, P).T)


def _run(
    x,
    w1e,
    b1,
    w2e,
    b2,
    wo,
    bo,
    d_in,
    d_h,
    d_out,
    n_cores=N_CORES,
    trace=False,
    mm_dtype=MM_DTYPE,
):
    b = x.shape[0]
    bc = b // n_cores

    nc = bacc.Bacc(
        "TRN2", target_bir_lowering=False, debug=False, num_devices=n_cores
    )
    _build(nc, d_in, d_h, d_out, bc, mm_dtype=mm_dtype)

    np_sdt = mybir.dt.np(BF16) if mm_dtype == "bf16" else np.float32

    def cvt(a):
        return np.ascontiguousarray(a.astype(np_sdt))

    shared = {
        "w1": cvt(_pack_lhsT(w1e, d_h, d_in)),
        "b1": _pack_bias(b1),
        "w2": cvt(_pack_lhsT(w2e, d_h, d_h)),
        "b2": _pack_bias(b2),
        "wo": cvt(_pack_out_panels(wo, d_out, d_h)),
        "bo": _pack_bias(bo),
    }
    in_maps = []
    for c in range(n_cores):
        xc = np.ascontiguousarray(x[c * bc : (c + 1) * bc].T).reshape(
            d_in // P, P, bc
        )
        in_maps.append({"xt": cvt(xc), **shared})

    res = run_bass_kernel_spmd(
        nc, in_maps, core_ids=list(range(n_cores)), trace=trace
    )
    outs = []
    for c in range(n_cores):
        outs.append(res.results[c]["out"].reshape(d_out, bc))
    full = np.concatenate(outs, axis=1)  # [d_out, B]
    return np.ascontiguousarray(full.T), res


def kernel(x, W1, b1, W2, b2, Wo, bo, mask1, mask2):
    x = np.asarray(x, dtype=np.float32)
    w1e = np.asarray(W1, dtype=np.float32) * _expand_mask(mask1)
    w2e = np.asarray(W2, dtype=np.float32) * _expand_mask(mask2)
    out, _ = _run(
        x,
        w1e,
        np.asarray(b1, np.float32),
        w2e,
        np.asarray(b2, np.float32),
        np.asarray(Wo, np.float32),
        np.asarray(bo, np.float32),
        d_in=2048,
        d_h=4096,
        d_out=1024,
    )
    return out
